# revision 33
# baseline (speedup 1.0000x reference)
"""Trainium2 Bass kernel for EnhancedGraphGenerator (GAT-style pairwise scorer).

Math (reference):
    h   = relu(x @ W1 + b1) @ W2 + b2                       # [N, H]
    e_i = h @ Wa1[:H]; e_j = h @ Wa1[H:]                    # [N, H]
    z   = relu(e_i[:,None,:] + e_j[None,:,:] + ba1)         # [N, N, H]
    s   = z . wa2 + ba2; s /= clip(t, .1, 5); s = (s+s.T)/2
    adj = sigmoid(s);  loss = 0.01 * mean(|adj|)

Distribution: row-sharded across 8 cores; h/e replicated (cheap), each core
computes both (i,j) and (j,i) raw scores for its 128 rows locally.

Device layout: h on partitions (64, duplicated to 128 for an i-pair), node
index on free dim. Pre-activation relu(e_jT + bias_col) is one fused
tensor_scalar/activation op over [128, 1024] in bf16 (DVE 4x mode); spread
over DVE/GPSIMD/ACT. The wa2 dot is a bf16 PE matmul with stationary
[128, 32] weights (wa2 in cols 0/1, zeros elsewhere -> psum rows 2..31
zeroed for free). tile_position col-tiling packs 4 i-pair blocks per PSUM
bank pair; pass A starts, pass B accumulates (same col group -> same bank
partitions, no cross-group has_written hazard). One ACT sigmoid per quad
evacuates into a persistent sig_all staging tile; 8 big strided DMAs write
the final adjacency rows.
"""

import sys

sys.path.insert(0, "/opt/trn_rl_repo")

import numpy as np

N, F, H = 1024, 512, 64
NCORES = 8
BLK = N // NCORES  # 128 rows per core
SPARSITY_WEIGHT = 0.01

_cache = {}

# producer engine schedule (weighted round robin): v=DVE, g=GPSIMD, a=ACT
N_PROD, N_ACT = 128, 32
PROD_PATTERN = "".join(
    "a" if (i * N_ACT) // N_PROD != ((i + 1) * N_ACT) // N_PROD else "v"
    for i in range(N_PROD))


def _build_program():
    import concourse.bacc as bacc
    import concourse.tile as tile
    from concourse import mybir

    f32 = mybir.dt.float32
    bf16 = mybir.dt.bfloat16
    AF = mybir.ActivationFunctionType
    OP = mybir.AluOpType

    nc = bacc.Bacc("TRN2", target_bir_lowering=False, debug=False)

    # ---- DRAM I/O ----
    xT_d = nc.dram_tensor("xT", [F, N], bf16, kind="ExternalInput")
    xbT_d = nc.dram_tensor("xbT", [F, BLK], bf16, kind="ExternalInput")
    w1_d = nc.dram_tensor("w1", [F, H], bf16, kind="ExternalInput")
    w2_d = nc.dram_tensor("w2", [H, H], bf16, kind="ExternalInput")
    wa1lo_d = nc.dram_tensor("wa1lo", [H, 128], bf16, kind="ExternalInput")
    wa1hi_d = nc.dram_tensor("wa1hi", [H, 128], bf16, kind="ExternalInput")
    wa2c_d = nc.dram_tensor("wa2c", [128, 32], bf16, kind="ExternalInput")
    b1_d = nc.dram_tensor("b1c", [H, 1], f32, kind="ExternalInput")
    b2_d = nc.dram_tensor("b2c", [H, 1], f32, kind="ExternalInput")
    ba12_d = nc.dram_tensor("ba12", [128, 1], f32, kind="ExternalInput")
    bacj_d = nc.dram_tensor("bacj", [128, 1], f32, kind="ExternalInput")
    baci_d = nc.dram_tensor("baci", [128, 1], f32, kind="ExternalInput")
    scol_d = nc.dram_tensor("scol", [128, 1], f32, kind="ExternalInput")
    sbcol_d = nc.dram_tensor("sbcol", [128, 1], f32, kind="ExternalInput")
    wcej_d = nc.dram_tensor("wcej", [H, 128], bf16, kind="ExternalInput")
    wcei_d = nc.dram_tensor("wcei", [H, 128], bf16, kind="ExternalInput")
    wc2_d = nc.dram_tensor("wc2", [H, 2], bf16, kind="ExternalInput")
    mc_d = nc.dram_tensor("mc", [128, 2], f32, kind="ExternalInput")
    sel8_d = nc.dram_tensor("sel8", [128, 32], bf16, kind="ExternalInput")
    sel2_d = nc.dram_tensor("sel2", [128, 32], bf16, kind="ExternalInput")
    adj_d = nc.dram_tensor("adj_blk", [BLK, N], f32, kind="ExternalOutput")

    with tile.TileContext(nc) as tc:
        with (
            tc.tile_pool(name="consts", bufs=1) as cp,
            tc.tile_pool(name="pre", bufs=12) as pp,
        ):
            ps0_cm = tc.tile_pool(name="psum0", bufs=2, space="PSUM")
            ps0 = ps0_cm.__enter__()
            # ---- const / input loads (ordered by first use; the two
            # DMA queues serialize, so gating transfers go first) ----
            w1t = cp.tile([128, 4 * H], bf16)
            for c in range(4):
                nc.sync.dma_start(w1t[:, c * H:(c + 1) * H],
                                  w1_d[c * 128:(c + 1) * 128, :])
            b1c = cp.tile([H, 1], f32)
            nc.gpsimd.dma_start(b1c[:], b1_d[:])
            wcej = cp.tile([H, 128], bf16)
            nc.gpsimd.dma_start(wcej[:], wcej_d[:])
            wcei = cp.tile([H, 128], bf16)
            nc.gpsimd.dma_start(wcei[:], wcei_d[:])
            xt = cp.tile([128, 4 * N], bf16)
            for c in range(4):
                for hh in range(2):
                    eng = nc.sync if (2 * c + hh) % 2 == 0 else nc.gpsimd
                    eng.dma_start(
                        xt[:, c * N + hh * 512:c * N + (hh + 1) * 512],
                        xT_d[c * 128:(c + 1) * 128, hh * 512:(hh + 1) * 512])
            xbt = cp.tile([128, 4 * BLK], bf16)
            for c in range(4):
                eng = nc.sync if c % 2 == 0 else nc.gpsimd
                eng.dma_start(xbt[:, c * BLK:(c + 1) * BLK],
                              xbT_d[c * 128:(c + 1) * 128, :])
            w2t = cp.tile([H, H], bf16)
            nc.sync.dma_start(w2t[:], w2_d[:])
            b2c = cp.tile([H, 1], f32)
            nc.gpsimd.dma_start(b2c[:], b2_d[:])
            wa1lo = cp.tile([H, 128], bf16)
            nc.sync.dma_start(wa1lo[:], wa1lo_d[:])
            wa1hi = cp.tile([H, 128], bf16)
            nc.gpsimd.dma_start(wa1hi[:], wa1hi_d[:])
            bacj = cp.tile([128, 1], f32)
            nc.sync.dma_start(bacj[:], bacj_d[:])
            baci = cp.tile([128, 1], f32)
            nc.gpsimd.dma_start(baci[:], baci_d[:])
            wa2c = cp.tile([128, 32], bf16)
            nc.sync.dma_start(wa2c[:], wa2c_d[:])
            ba12 = cp.tile([128, 1], f32)
            nc.gpsimd.dma_start(ba12[:], ba12_d[:])
            scol = cp.tile([128, 1], f32)
            nc.sync.dma_start(scol[:], scol_d[:])
            sbcol = cp.tile([128, 1], f32)
            nc.gpsimd.dma_start(sbcol[:], sbcol_d[:])
            wc2 = cp.tile([H, 2], bf16)
            nc.sync.dma_start(wc2[:], wc2_d[:])
            mc = cp.tile([128, 2], f32)
            nc.gpsimd.dma_start(mc[:], mc_d[:])
            sel8 = cp.tile([128, 32], bf16)
            nc.sync.dma_start(sel8[:], sel8_d[:])
            sel2 = cp.tile([128, 32], bf16)
            nc.gpsimd.dma_start(sel2[:], sel2_d[:])

            # ---- stage 0, i-side (this core's 128 rows, from xbT) ----
            h1Tb = cp.tile([H, BLK], bf16)
            hTb = cp.tile([H, BLK], bf16)
            eiTb2p = cp.tile([128, BLK], f32)
            ejTb2p = cp.tile([128, BLK], f32)
            psb1 = ps0.tile([H, BLK], f32, tag="s0")
            for c in range(4):
                nc.tensor.matmul(psb1[:], w1t[:, c * H:(c + 1) * H],
                                 xbt[:, c * BLK:(c + 1) * BLK],
                                 start=(c == 0), stop=(c == 3))
            nc.scalar.activation(h1Tb[:], psb1[:], AF.Relu, bias=b1c[:])
            psb2 = ps0.tile([H, BLK], f32, tag="s0")
            nc.tensor.matmul(psb2[:], w2t[:], h1Tb[:], start=True, stop=True)
            nc.scalar.activation(hTb[:], psb2[:], AF.Identity, bias=b2c[:])
            psbe = ps0.tile([128, BLK], f32, tag="s0")
            nc.tensor.matmul(psbe[:], wa1lo[:], hTb[:], start=True, stop=True)
            nc.scalar.activation(eiTb2p[:], psbe[:], AF.Identity, bias=bacj[:])
            psbe2 = ps0.tile([128, BLK], f32, tag="s0")
            nc.tensor.matmul(psbe2[:], wa1hi[:], hTb[:], start=True, stop=True)
            nc.scalar.activation(ejTb2p[:], psbe2[:], AF.Identity, bias=baci[:])

            # bias columns: biasA[:, b] = [e_i(2b)+ba1 ; e_i(2b+1)+ba1]
            # (positive for ACT relu-form, negated for DVE max-form:
            #  relu(e+b) = max(e,-b) + b, the +b folds into a per-row
            #  constant applied at the final sigmoid)
            biasA = cp.tile([128, 64], f32)
            biasB = cp.tile([128, 64], f32)
            nc.vector.tensor_copy(biasA[0:64, :], eiTb2p[0:64, 0:128:2])
            nc.vector.tensor_copy(biasA[64:128, :], eiTb2p[64:128, 1:128:2])
            nc.vector.tensor_copy(biasB[0:64, :], ejTb2p[0:64, 0:128:2])
            nc.vector.tensor_copy(biasB[64:128, :], ejTb2p[64:128, 1:128:2])
            nbiasA = cp.tile([128, 64], f32)
            nbiasB = cp.tile([128, 64], f32)
            nc.vector.tensor_scalar(nbiasA[:], biasA[:], -1.0, None, OP.mult)
            nc.vector.tensor_scalar(nbiasB[:], biasB[:], -1.0, None, OP.mult)

            # per-row correction C_i = sum_h wa2[h]*(e_i+ba1) / (e_j+ba1),
            # masked by which pass used the max-form, times scale, plus
            # sigmoid bias -> final per-partition bias column
            psc2 = ps0.tile([128, 2], f32, tag="s0")
            nc.tensor.matmul(psc2[:], hTb[:], wc2[:], start=True, stop=True)
            cc2 = cp.tile([128, 2], f32)
            nc.vector.tensor_mul(cc2[:], psc2[:], mc[:])
            fb1 = cp.tile([128, 1], f32)
            nc.vector.tensor_add(fb1[:], cc2[:, 0:1], cc2[:, 1:2])
            fb2 = cp.tile([128, 1], f32)
            nc.vector.tensor_scalar(fb2[:], fb1[:], scol[:], sbcol[:],
                                    OP.mult, OP.add)

            # ---- stage 0, j-side: e_jT / e_iT over all N, duplicated 2x on
            # partitions (rows 0-63 and 64-127 both hold the 64 h-dims) ----
            h1T = cp.tile([H, N], bf16)
            ejT2 = cp.tile([128, N], bf16)
            eiT2 = cp.tile([128, N], bf16)
            for jh in range(2):
                s = slice(512 * jh, 512 * (jh + 1))
                ps1 = ps0.tile([H, 512], f32, tag="s0")
                for c in range(4):
                    nc.tensor.matmul(ps1[:], w1t[:, c * H:(c + 1) * H],
                                     xt[:, c * N + 512 * jh: c * N + 512 * (jh + 1)],
                                     start=(c == 0), stop=(c == 3))
                nc.scalar.activation(h1T[:, s], ps1[:], AF.Relu, bias=b1c[:])
                # e_jT = (W2 @ Wa1_hi_dup).T @ h1T  (+b2-term folded into the
                # bias columns on the other side of the pre-act sum)
                psj = ps0.tile([128, 512], f32, tag="s0")
                nc.tensor.matmul(psj[:], wcej[:], h1T[:, s], start=True, stop=True)
                nc.vector.tensor_copy(ejT2[:, s], psj[:])
                psi = ps0.tile([128, 512], f32, tag="s0")
                nc.tensor.matmul(psi[:], wcei[:], h1T[:, s], start=True, stop=True)
                nc.vector.tensor_copy(eiT2[:, s], psi[:])

            ps0_cm.__exit__(None, None, None)
            psM_cm = tc.tile_pool(name="psumM", bufs=2, space="PSUM")
            psM = psM_cm.__enter__()
            psC_cm = tc.tile_pool(name="psumC", bufs=2, space="PSUM")
            psC = psC_cm.__enter__()

            # ---- main loop: 16 quads x 4 col-groups x (A,B) passes ----
            prod_idx = 0

            def producer(out_t, in_t, pos_col, neg_col):
                nonlocal prod_idx
                kind = PROD_PATTERN[prod_idx % len(PROD_PATTERN)]
                if kind == "a":
                    nc.scalar.activation(out_t[:], in_t[:], AF.Relu,
                                         bias=pos_col)
                else:
                    nc.vector.tensor_scalar(out_t[:], in_t[:], neg_col, None,
                                            OP.max)
                prod_idx += 1

            # compaction: stage-1 gathers each quad's 8 scattered psum rows
            # (partitions {32g, 32g+1}) to rows 32c..32c+8 of CP[a] where
            # q = 4a + c; stage-2 gathers CP's 4x8 rows into 32 contiguous
            # rows per col group -> fully i-ordered CP2 [128, 1024]
            CS = cp.tile([128, 4 * N], bf16)
            adj_s = cp.tile([128, N], f32)
            cp2 = psC.tile([128, N], f32, tag="cp", name="cp2")
            state = {"cpa": None, "ready_a": None}

            def finalize_a(a):
                # stage-2 compaction + sigmoid + output DMA for one group of
                # 32 adjacency rows (fully i-ordered)
                for jh in range(2):
                    s = slice(512 * jh, 512 * (jh + 1))
                    nc.tensor.matmul(cp2[32 * a:32 * a + 32, s], sel2[:],
                                     CS[:, a * N + 512 * jh:
                                         a * N + 512 * (jh + 1)],
                                     start=True, stop=True,
                                     tile_position=(0, 32 * a),
                                     skip_group_check=True)
                r = slice(32 * a, 32 * a + 32)
                nc.scalar.activation(adj_s[r, :], cp2[r, :], AF.Sigmoid,
                                     bias=fb2[r, :], scale=scol[r, :])
                eng = nc.sync if a % 2 == 0 else nc.gpsimd
                eng.dma_start(adj_d[r, :], adj_s[r, :])

            def epilogue(q, pq):
                # runs one quad behind the producer/MM front so the FIFO
                # engine queues never stall on cross-engine dependencies
                a, c = q // 4, q % 4
                sig = pp.tile([128, N], bf16, tag="sig")
                if q % 2 == 0:
                    nc.vector.tensor_copy(sig[:], pq[:])
                else:
                    nc.scalar.copy(sig[:], pq[:])
                if c == 0:
                    state["cpa"] = psC.tile([128, N], f32, tag="cp", name="cpa")
                cpa = state["cpa"]
                for jh in range(2):
                    s = slice(512 * jh, 512 * (jh + 1))
                    nc.tensor.matmul(cpa[32 * c:32 * c + 32, s], sel8[:],
                                     sig[:, s], start=True, stop=True,
                                     tile_position=(0, 32 * c),
                                     skip_group_check=True)
                if state["ready_a"] is not None:
                    finalize_a(state["ready_a"])
                    state["ready_a"] = None
                if c == 3:
                    if a % 2 == 0:
                        nc.vector.tensor_copy(CS[:, a * N:(a + 1) * N], cpa[:])
                    else:
                        nc.scalar.copy(CS[:, a * N:(a + 1) * N], cpa[:])
                    state["ready_a"] = a

            pending = None
            for q in range(16):
                pq = psM.tile([128, N], f32, tag="pq")
                pres = []
                for g in range(4):
                    b = 4 * q + g
                    preA = pp.tile([128, N], bf16, tag="pre")
                    producer(preA, ejT2, biasA[:, b:b + 1], nbiasA[:, b:b + 1])
                    preB = pp.tile([128, N], bf16, tag="pre")
                    producer(preB, eiT2, biasB[:, b:b + 1], nbiasB[:, b:b + 1])
                    pres.append((preA, preB))
                if pending is not None:
                    epilogue(*pending)
                # sim's group tracker is partition-base blind -> false
                # collisions across col groups; HW has_written is
                # per-partition and each MM consumes its whole zero
                # region, so skipping the check is sound here
                for jh in range(2):
                    s = slice(512 * jh, 512 * (jh + 1))
                    for g in range(4):
                        nc.tensor.matmul(pq[32 * g:32 * g + 32, s], wa2c[:],
                                         pres[g][0][:, s], start=True,
                                         stop=False, tile_position=(0, 32 * g),
                                         skip_group_check=True)
                for jh in range(2):
                    s = slice(512 * jh, 512 * (jh + 1))
                    for g in range(4):
                        nc.tensor.matmul(pq[32 * g:32 * g + 32, s], wa2c[:],
                                         pres[g][1][:, s], start=False,
                                         stop=True, tile_position=(0, 32 * g),
                                         skip_group_check=True)
                pending = (q, pq)
            epilogue(*pending)

            if state["ready_a"] is not None:
                finalize_a(state["ready_a"])
            psC_cm.__exit__(None, None, None)
            psM_cm.__exit__(None, None, None)

    nc.compile()
    return nc


def _host_prep(node_features, W1, b1, W2, b2, Wa1, ba1, wa2, ba2, temperature):
    """Host-side input layout prep (transposes / tiling / scalar folding)."""
    import ml_dtypes

    x = np.asarray(node_features, np.float32)
    W1 = np.asarray(W1, np.float32)
    W2 = np.asarray(W2, np.float32)
    Wa1 = np.asarray(Wa1, np.float32)
    b1 = np.asarray(b1, np.float32)
    b2 = np.asarray(b2, np.float32)
    ba1 = np.asarray(ba1, np.float32)
    wa2 = np.asarray(wa2, np.float32)
    ba2 = np.float32(ba2)
    t = float(np.clip(np.float32(temperature), 0.1, 5.0))

    xT = np.ascontiguousarray(x.T)                           # [F, N]
    wa1lo = np.ascontiguousarray(np.tile(Wa1[:H], (1, 2)))   # [64, 128]
    wa1hi = np.ascontiguousarray(np.tile(Wa1[H:], (1, 2)))   # [64, 128]
    wa2c = np.zeros((128, 32), np.float32)
    wa2c[0:64, 0] = wa2
    wa2c[64:128, 1] = wa2
    ba12 = np.tile(ba1, 2).reshape(128, 1)
    scol = np.full((128, 1), 0.5 / t, np.float32)
    sbcol = np.full((128, 1), ba2 / t, np.float32)

    wcej = np.ascontiguousarray(W2 @ np.tile(Wa1[H:], (1, 2)))  # [64, 128]
    wcei = np.ascontiguousarray(W2 @ np.tile(Wa1[:H], (1, 2)))
    cj = np.tile(Wa1[H:].T @ b2, 2)    # b2-term of e_j, dup layout
    ci = np.tile(Wa1[:H].T @ b2, 2)
    bacj = (np.tile(ba1, 2) + cj).reshape(128, 1).astype(np.float32)
    baci = (np.tile(ba1, 2) + ci).reshape(128, 1).astype(np.float32)
    wcomb2 = np.stack([Wa1[:H] @ wa2, Wa1[H:] @ wa2], axis=1)  # [64, 2]
    cba = float(wa2 @ ba1)
    maskA = np.array([1.0 if PROD_PATTERN[(2 * b) % len(PROD_PATTERN)] == "v"
                      else 0.0 for b in range(64)], np.float32)
    maskB = np.array([1.0 if PROD_PATTERN[(2 * b + 1) % len(PROD_PATTERN)] == "v"
                      else 0.0 for b in range(64)], np.float32)
    mcol = np.zeros((128, 2), np.float32)
    mcol[:, 0] = np.repeat(maskA, 2)
    mcol[:, 1] = np.repeat(maskB, 2)
    ccj = float(wa2 @ (Wa1[H:].T @ b2))   # b2-term of pass-A bias fold
    cci = float(wa2 @ (Wa1[:H].T @ b2))   # b2-term of pass-B bias fold
    sbcol2 = np.float32(ba2 / t) + np.float32(0.5 / t) * (
        (cba + ccj) * np.repeat(maskA, 2) + (cba + cci) * np.repeat(maskB, 2))

    sel8 = np.zeros((128, 32), np.float32)
    for m in range(8):
        sel8[32 * (m // 2) + (m % 2), m] = 1.0
    sel2 = np.zeros((128, 32), np.float32)
    for c in range(4):
        for mp in range(8):
            sel2[32 * c + mp, 8 * c + mp] = 1.0

    common = {
        "sel8": sel8.astype(ml_dtypes.bfloat16),
        "sel2": sel2.astype(ml_dtypes.bfloat16),
        "xT": xT.astype(ml_dtypes.bfloat16),
        "w1": W1.astype(ml_dtypes.bfloat16),
        "w2": W2.astype(ml_dtypes.bfloat16),
        "wa1lo": wa1lo.astype(ml_dtypes.bfloat16),
        "wa1hi": wa1hi.astype(ml_dtypes.bfloat16),
        "wa2c": wa2c.astype(ml_dtypes.bfloat16),
        "b1c": b1.reshape(H, 1),
        "b2c": b2.reshape(H, 1),
        "ba12": ba12.astype(np.float32),
        "scol": scol,
        "sbcol": sbcol2.reshape(128, 1).astype(np.float32),
        "wc2": wcomb2.astype(ml_dtypes.bfloat16),
        "wcej": wcej.astype(ml_dtypes.bfloat16),
        "wcei": wcei.astype(ml_dtypes.bfloat16),
        "bacj": bacj,
        "baci": baci,
        "mc": mcol,
    }
    in_maps = []
    for c in range(NCORES):
        m = dict(common)
        m["xbT"] = np.ascontiguousarray(
            xT[:, c * BLK:(c + 1) * BLK]).astype(ml_dtypes.bfloat16)
        in_maps.append(m)
    return in_maps


def kernel(node_features, W1, b1, W2, b2, Wa1, ba1, wa2, ba2, temperature):
    from concourse.bass_utils import run_bass_kernel_spmd

    if "nc" not in _cache:
        _cache["nc"] = _build_program()
    nc = _cache["nc"]

    in_maps = _host_prep(node_features, W1, b1, W2, b2, Wa1, ba1, wa2, ba2,
                         temperature)
    res = run_bass_kernel_spmd(nc, in_maps, list(range(NCORES)))
    adj = np.concatenate([res.results[c]["adj_blk"] for c in range(NCORES)],
                         axis=0)
    loss = np.float32(SPARSITY_WEIGHT) * np.mean(np.abs(adj), dtype=np.float32)
    return adj, np.float32(loss)


# revision 34
# speedup vs baseline: 1.0265x; 1.0265x over previous
"""Trainium2 Bass kernel for EnhancedGraphGenerator (GAT-style pairwise scorer).

Math (reference):
    h   = relu(x @ W1 + b1) @ W2 + b2                       # [N, H]
    e_i = h @ Wa1[:H]; e_j = h @ Wa1[H:]                    # [N, H]
    z   = relu(e_i[:,None,:] + e_j[None,:,:] + ba1)         # [N, N, H]
    s   = z . wa2 + ba2; s /= clip(t, .1, 5); s = (s+s.T)/2
    adj = sigmoid(s);  loss = 0.01 * mean(|adj|)

Distribution: row-sharded across 8 cores; h/e replicated (cheap), each core
computes both (i,j) and (j,i) raw scores for its 128 rows locally.

Device layout: h on partitions (64, duplicated to 128 for an i-pair), node
index on free dim. Pre-activation relu(e_jT + bias_col) is one fused
tensor_scalar/activation op over [128, 1024] in bf16 (DVE 4x mode); spread
over DVE/GPSIMD/ACT. The wa2 dot is a bf16 PE matmul with stationary
[128, 32] weights (wa2 in cols 0/1, zeros elsewhere -> psum rows 2..31
zeroed for free). tile_position col-tiling packs 4 i-pair blocks per PSUM
bank pair; pass A starts, pass B accumulates (same col group -> same bank
partitions, no cross-group has_written hazard). One ACT sigmoid per quad
evacuates into a persistent sig_all staging tile; 8 big strided DMAs write
the final adjacency rows.
"""

import sys

sys.path.insert(0, "/opt/trn_rl_repo")

import numpy as np

N, F, H = 1024, 512, 64
NCORES = 8
BLK = N // NCORES  # 128 rows per core
SPARSITY_WEIGHT = 0.01

_cache = {}

# producer engine schedule (weighted round robin): v=DVE, g=GPSIMD, a=ACT
N_PROD, N_ACT = 128, 32
PROD_PATTERN = "".join(
    "a" if (i * N_ACT) // N_PROD != ((i + 1) * N_ACT) // N_PROD else "v"
    for i in range(N_PROD))


def _build_program():
    import concourse.bacc as bacc
    import concourse.tile as tile
    from concourse import mybir

    f32 = mybir.dt.float32
    bf16 = mybir.dt.bfloat16
    AF = mybir.ActivationFunctionType
    OP = mybir.AluOpType

    nc = bacc.Bacc("TRN2", target_bir_lowering=False, debug=False)

    # ---- DRAM I/O ----
    xT_d = nc.dram_tensor("xT", [F, N], bf16, kind="ExternalInput")
    xbT_d = nc.dram_tensor("xbT", [F, BLK], bf16, kind="ExternalInput")
    w1_d = nc.dram_tensor("w1", [F, H], bf16, kind="ExternalInput")
    w2_d = nc.dram_tensor("w2", [H, H], bf16, kind="ExternalInput")
    wa1lo_d = nc.dram_tensor("wa1lo", [H, 128], bf16, kind="ExternalInput")
    wa1hi_d = nc.dram_tensor("wa1hi", [H, 128], bf16, kind="ExternalInput")
    wa2c_d = nc.dram_tensor("wa2c", [128, 32], bf16, kind="ExternalInput")
    b1_d = nc.dram_tensor("b1c", [H, 1], f32, kind="ExternalInput")
    b2_d = nc.dram_tensor("b2c", [H, 1], f32, kind="ExternalInput")
    ba12_d = nc.dram_tensor("ba12", [128, 1], f32, kind="ExternalInput")
    bacj_d = nc.dram_tensor("bacj", [128, 1], f32, kind="ExternalInput")
    baci_d = nc.dram_tensor("baci", [128, 1], f32, kind="ExternalInput")
    scol_d = nc.dram_tensor("scol", [128, 1], f32, kind="ExternalInput")
    sbcol_d = nc.dram_tensor("sbcol", [128, 1], f32, kind="ExternalInput")
    wcej_d = nc.dram_tensor("wcej", [H, 128], bf16, kind="ExternalInput")
    wcei_d = nc.dram_tensor("wcei", [H, 128], bf16, kind="ExternalInput")
    wc2_d = nc.dram_tensor("wc2", [H, 2], bf16, kind="ExternalInput")
    mc_d = nc.dram_tensor("mc", [128, 2], f32, kind="ExternalInput")
    sel8_d = nc.dram_tensor("sel8", [128, 32], bf16, kind="ExternalInput")
    sel2_d = nc.dram_tensor("sel2", [128, 32], bf16, kind="ExternalInput")
    adj_d = nc.dram_tensor("adj_blk", [BLK, N], f32, kind="ExternalOutput")

    with tile.TileContext(nc) as tc:
        with (
            tc.tile_pool(name="consts", bufs=1) as cp,
            tc.tile_pool(name="pre", bufs=12) as pp,
        ):
            ps0_cm = tc.tile_pool(name="psum0", bufs=2, space="PSUM")
            ps0 = ps0_cm.__enter__()
            # ---- const / input loads (ordered by first use; the two
            # DMA queues serialize, so gating transfers go first) ----
            w1t = cp.tile([128, 4 * H], bf16)
            for c in range(4):
                nc.sync.dma_start(w1t[:, c * H:(c + 1) * H],
                                  w1_d[c * 128:(c + 1) * 128, :])
            b1c = cp.tile([H, 1], f32)
            nc.gpsimd.dma_start(b1c[:], b1_d[:])
            wcej = cp.tile([H, 128], bf16)
            nc.gpsimd.dma_start(wcej[:], wcej_d[:])
            wcei = cp.tile([H, 128], bf16)
            nc.gpsimd.dma_start(wcei[:], wcei_d[:])
            xt = cp.tile([128, 4 * N], bf16)
            for c in range(4):
                for hh in range(2):
                    eng = nc.sync if (2 * c + hh) % 2 == 0 else nc.gpsimd
                    eng.dma_start(
                        xt[:, c * N + hh * 512:c * N + (hh + 1) * 512],
                        xT_d[c * 128:(c + 1) * 128, hh * 512:(hh + 1) * 512])
            xbt = cp.tile([128, 4 * BLK], bf16)
            for c in range(4):
                eng = nc.sync if c % 2 == 0 else nc.gpsimd
                eng.dma_start(xbt[:, c * BLK:(c + 1) * BLK],
                              xbT_d[c * 128:(c + 1) * 128, :])
            w2t = cp.tile([H, H], bf16)
            nc.sync.dma_start(w2t[:], w2_d[:])
            b2c = cp.tile([H, 1], f32)
            nc.gpsimd.dma_start(b2c[:], b2_d[:])
            wa1lo = cp.tile([H, 128], bf16)
            nc.sync.dma_start(wa1lo[:], wa1lo_d[:])
            wa1hi = cp.tile([H, 128], bf16)
            nc.gpsimd.dma_start(wa1hi[:], wa1hi_d[:])
            bacj = cp.tile([128, 1], f32)
            nc.sync.dma_start(bacj[:], bacj_d[:])
            baci = cp.tile([128, 1], f32)
            nc.gpsimd.dma_start(baci[:], baci_d[:])
            wa2c = cp.tile([128, 32], bf16)
            nc.sync.dma_start(wa2c[:], wa2c_d[:])
            ba12 = cp.tile([128, 1], f32)
            nc.gpsimd.dma_start(ba12[:], ba12_d[:])
            scol = cp.tile([128, 1], f32)
            nc.sync.dma_start(scol[:], scol_d[:])
            sbcol = cp.tile([128, 1], f32)
            nc.gpsimd.dma_start(sbcol[:], sbcol_d[:])
            wc2 = cp.tile([H, 2], bf16)
            nc.sync.dma_start(wc2[:], wc2_d[:])
            mc = cp.tile([128, 2], f32)
            nc.gpsimd.dma_start(mc[:], mc_d[:])
            sel8 = cp.tile([128, 32], bf16)
            nc.sync.dma_start(sel8[:], sel8_d[:])
            sel2 = cp.tile([128, 32], bf16)
            nc.gpsimd.dma_start(sel2[:], sel2_d[:])

            # ---- stage 0, j-side: e_jT / e_iT over all N, duplicated 2x on
            # partitions (rows 0-63 and 64-127 both hold the 64 h-dims) ----
            h1T = cp.tile([H, N], bf16)
            ejT2 = cp.tile([128, N], bf16)
            eiT2 = cp.tile([128, N], bf16)
            for jh in range(2):
                s = slice(512 * jh, 512 * (jh + 1))
                ps1 = ps0.tile([H, 512], f32, tag="s0")
                for c in range(4):
                    nc.tensor.matmul(ps1[:], w1t[:, c * H:(c + 1) * H],
                                     xt[:, c * N + 512 * jh: c * N + 512 * (jh + 1)],
                                     start=(c == 0), stop=(c == 3))
                nc.scalar.activation(h1T[:, s], ps1[:], AF.Relu, bias=b1c[:])
                # e_jT = (W2 @ Wa1_hi_dup).T @ h1T  (+b2-term folded into the
                # bias columns on the other side of the pre-act sum)
                psj = ps0.tile([128, 512], f32, tag="s0")
                nc.tensor.matmul(psj[:], wcej[:], h1T[:, s], start=True, stop=True)
                nc.vector.tensor_copy(ejT2[:, s], psj[:])
                psi = ps0.tile([128, 512], f32, tag="s0")
                nc.tensor.matmul(psi[:], wcei[:], h1T[:, s], start=True, stop=True)
                nc.vector.tensor_copy(eiT2[:, s], psi[:])

            # ---- stage 0, i-side (this core's 128 rows, from xbT) ----
            h1Tb = cp.tile([H, BLK], bf16)
            hTb = cp.tile([H, BLK], bf16)
            eiTb2p = cp.tile([128, BLK], f32)
            ejTb2p = cp.tile([128, BLK], f32)
            psb1 = ps0.tile([H, BLK], f32, tag="s0")
            for c in range(4):
                nc.tensor.matmul(psb1[:], w1t[:, c * H:(c + 1) * H],
                                 xbt[:, c * BLK:(c + 1) * BLK],
                                 start=(c == 0), stop=(c == 3))
            nc.scalar.activation(h1Tb[:], psb1[:], AF.Relu, bias=b1c[:])
            psb2 = ps0.tile([H, BLK], f32, tag="s0")
            nc.tensor.matmul(psb2[:], w2t[:], h1Tb[:], start=True, stop=True)
            nc.scalar.activation(hTb[:], psb2[:], AF.Identity, bias=b2c[:])
            psbe = ps0.tile([128, BLK], f32, tag="s0")
            nc.tensor.matmul(psbe[:], wa1lo[:], hTb[:], start=True, stop=True)
            nc.scalar.activation(eiTb2p[:], psbe[:], AF.Identity, bias=bacj[:])
            psbe2 = ps0.tile([128, BLK], f32, tag="s0")
            nc.tensor.matmul(psbe2[:], wa1hi[:], hTb[:], start=True, stop=True)
            nc.scalar.activation(ejTb2p[:], psbe2[:], AF.Identity, bias=baci[:])

            # bias columns: biasA[:, b] = [e_i(2b)+ba1 ; e_i(2b+1)+ba1]
            # (positive for ACT relu-form, negated for DVE max-form:
            #  relu(e+b) = max(e,-b) + b, the +b folds into a per-row
            #  constant applied at the final sigmoid)
            biasA = cp.tile([128, 64], f32)
            biasB = cp.tile([128, 64], f32)
            nc.vector.tensor_copy(biasA[0:64, :], eiTb2p[0:64, 0:128:2])
            nc.vector.tensor_copy(biasA[64:128, :], eiTb2p[64:128, 1:128:2])
            nc.vector.tensor_copy(biasB[0:64, :], ejTb2p[0:64, 0:128:2])
            nc.vector.tensor_copy(biasB[64:128, :], ejTb2p[64:128, 1:128:2])
            nbiasA = cp.tile([128, 64], f32)
            nbiasB = cp.tile([128, 64], f32)
            nc.vector.tensor_scalar(nbiasA[:], biasA[:], -1.0, None, OP.mult)
            nc.vector.tensor_scalar(nbiasB[:], biasB[:], -1.0, None, OP.mult)

            # per-row correction C_i = sum_h wa2[h]*(e_i+ba1) / (e_j+ba1),
            # masked by which pass used the max-form, times scale, plus
            # sigmoid bias -> final per-partition bias column
            psc2 = ps0.tile([128, 2], f32, tag="s0")
            nc.tensor.matmul(psc2[:], hTb[:], wc2[:], start=True, stop=True)
            cc2 = cp.tile([128, 2], f32)
            nc.vector.tensor_mul(cc2[:], psc2[:], mc[:])
            fb1 = cp.tile([128, 1], f32)
            nc.vector.tensor_add(fb1[:], cc2[:, 0:1], cc2[:, 1:2])
            fb2 = cp.tile([128, 1], f32)
            nc.vector.tensor_scalar(fb2[:], fb1[:], scol[:], sbcol[:],
                                    OP.mult, OP.add)

            ps0_cm.__exit__(None, None, None)
            psM_cm = tc.tile_pool(name="psumM", bufs=2, space="PSUM")
            psM = psM_cm.__enter__()
            psC_cm = tc.tile_pool(name="psumC", bufs=2, space="PSUM")
            psC = psC_cm.__enter__()

            # ---- main loop: 16 quads x 4 col-groups x (A,B) passes ----
            prod_idx = 0

            def producer(out_t, in_t, pos_col, neg_col):
                nonlocal prod_idx
                kind = PROD_PATTERN[prod_idx % len(PROD_PATTERN)]
                if kind == "a":
                    nc.scalar.activation(out_t[:], in_t[:], AF.Relu,
                                         bias=pos_col)
                else:
                    nc.vector.tensor_scalar(out_t[:], in_t[:], neg_col, None,
                                            OP.max)
                prod_idx += 1

            # compaction: stage-1 gathers each quad's 8 scattered psum rows
            # (partitions {32g, 32g+1}) to rows 32c..32c+8 of CP[a] where
            # q = 4a + c; stage-2 gathers CP's 4x8 rows into 32 contiguous
            # rows per col group -> fully i-ordered CP2 [128, 1024]
            CS = cp.tile([128, 4 * N], bf16)
            adj_s = cp.tile([128, N], f32)
            cp2 = psC.tile([128, N], f32, tag="cp", name="cp2")
            state = {"cpa": None, "ready_a": None}

            def finalize_a(a):
                # stage-2 compaction + sigmoid + output DMA for one group of
                # 32 adjacency rows (fully i-ordered)
                for jh in range(2):
                    s = slice(512 * jh, 512 * (jh + 1))
                    nc.tensor.matmul(cp2[32 * a:32 * a + 32, s], sel2[:],
                                     CS[:, a * N + 512 * jh:
                                         a * N + 512 * (jh + 1)],
                                     start=True, stop=True,
                                     tile_position=(0, 32 * a),
                                     skip_group_check=True)
                r = slice(32 * a, 32 * a + 32)
                nc.scalar.activation(adj_s[r, :], cp2[r, :], AF.Sigmoid,
                                     bias=fb2[r, :], scale=scol[r, :])
                eng = nc.sync if a % 2 == 0 else nc.gpsimd
                eng.dma_start(adj_d[r, :], adj_s[r, :])

            def epilogue(q, pq):
                # runs one quad behind the producer/MM front so the FIFO
                # engine queues never stall on cross-engine dependencies
                a, c = q // 4, q % 4
                sig = pp.tile([128, N], bf16, tag="sig")
                if q % 2 == 0:
                    nc.vector.tensor_copy(sig[:], pq[:])
                else:
                    nc.scalar.copy(sig[:], pq[:])
                if c == 0:
                    state["cpa"] = psC.tile([128, N], f32, tag="cp", name="cpa")
                cpa = state["cpa"]
                for jh in range(2):
                    s = slice(512 * jh, 512 * (jh + 1))
                    nc.tensor.matmul(cpa[32 * c:32 * c + 32, s], sel8[:],
                                     sig[:, s], start=True, stop=True,
                                     tile_position=(0, 32 * c),
                                     skip_group_check=True)
                if state["ready_a"] is not None:
                    finalize_a(state["ready_a"])
                    state["ready_a"] = None
                if c == 3:
                    if a % 2 == 0:
                        nc.vector.tensor_copy(CS[:, a * N:(a + 1) * N], cpa[:])
                    else:
                        nc.scalar.copy(CS[:, a * N:(a + 1) * N], cpa[:])
                    state["ready_a"] = a

            pending = None
            for q in range(16):
                pq = psM.tile([128, N], f32, tag="pq")
                pres = []
                for g in range(4):
                    b = 4 * q + g
                    preA = pp.tile([128, N], bf16, tag="pre")
                    producer(preA, ejT2, biasA[:, b:b + 1], nbiasA[:, b:b + 1])
                    preB = pp.tile([128, N], bf16, tag="pre")
                    producer(preB, eiT2, biasB[:, b:b + 1], nbiasB[:, b:b + 1])
                    pres.append((preA, preB))
                if pending is not None:
                    epilogue(*pending)
                # sim's group tracker is partition-base blind -> false
                # collisions across col groups; HW has_written is
                # per-partition and each MM consumes its whole zero
                # region, so skipping the check is sound here
                for jh in range(2):
                    s = slice(512 * jh, 512 * (jh + 1))
                    for g in range(4):
                        nc.tensor.matmul(pq[32 * g:32 * g + 32, s], wa2c[:],
                                         pres[g][0][:, s], start=True,
                                         stop=False, tile_position=(0, 32 * g),
                                         skip_group_check=True)
                for jh in range(2):
                    s = slice(512 * jh, 512 * (jh + 1))
                    for g in range(4):
                        nc.tensor.matmul(pq[32 * g:32 * g + 32, s], wa2c[:],
                                         pres[g][1][:, s], start=False,
                                         stop=True, tile_position=(0, 32 * g),
                                         skip_group_check=True)
                pending = (q, pq)
            epilogue(*pending)

            if state["ready_a"] is not None:
                finalize_a(state["ready_a"])
            psC_cm.__exit__(None, None, None)
            psM_cm.__exit__(None, None, None)

    nc.compile()
    return nc


def _host_prep(node_features, W1, b1, W2, b2, Wa1, ba1, wa2, ba2, temperature):
    """Host-side input layout prep (transposes / tiling / scalar folding)."""
    import ml_dtypes

    x = np.asarray(node_features, np.float32)
    W1 = np.asarray(W1, np.float32)
    W2 = np.asarray(W2, np.float32)
    Wa1 = np.asarray(Wa1, np.float32)
    b1 = np.asarray(b1, np.float32)
    b2 = np.asarray(b2, np.float32)
    ba1 = np.asarray(ba1, np.float32)
    wa2 = np.asarray(wa2, np.float32)
    ba2 = np.float32(ba2)
    t = float(np.clip(np.float32(temperature), 0.1, 5.0))

    xT = np.ascontiguousarray(x.T)                           # [F, N]
    wa1lo = np.ascontiguousarray(np.tile(Wa1[:H], (1, 2)))   # [64, 128]
    wa1hi = np.ascontiguousarray(np.tile(Wa1[H:], (1, 2)))   # [64, 128]
    wa2c = np.zeros((128, 32), np.float32)
    wa2c[0:64, 0] = wa2
    wa2c[64:128, 1] = wa2
    ba12 = np.tile(ba1, 2).reshape(128, 1)
    scol = np.full((128, 1), 0.5 / t, np.float32)
    sbcol = np.full((128, 1), ba2 / t, np.float32)

    wcej = np.ascontiguousarray(W2 @ np.tile(Wa1[H:], (1, 2)))  # [64, 128]
    wcei = np.ascontiguousarray(W2 @ np.tile(Wa1[:H], (1, 2)))
    cj = np.tile(Wa1[H:].T @ b2, 2)    # b2-term of e_j, dup layout
    ci = np.tile(Wa1[:H].T @ b2, 2)
    bacj = (np.tile(ba1, 2) + cj).reshape(128, 1).astype(np.float32)
    baci = (np.tile(ba1, 2) + ci).reshape(128, 1).astype(np.float32)
    wcomb2 = np.stack([Wa1[:H] @ wa2, Wa1[H:] @ wa2], axis=1)  # [64, 2]
    cba = float(wa2 @ ba1)
    maskA = np.array([1.0 if PROD_PATTERN[(2 * b) % len(PROD_PATTERN)] == "v"
                      else 0.0 for b in range(64)], np.float32)
    maskB = np.array([1.0 if PROD_PATTERN[(2 * b + 1) % len(PROD_PATTERN)] == "v"
                      else 0.0 for b in range(64)], np.float32)
    mcol = np.zeros((128, 2), np.float32)
    mcol[:, 0] = np.repeat(maskA, 2)
    mcol[:, 1] = np.repeat(maskB, 2)
    ccj = float(wa2 @ (Wa1[H:].T @ b2))   # b2-term of pass-A bias fold
    cci = float(wa2 @ (Wa1[:H].T @ b2))   # b2-term of pass-B bias fold
    sbcol2 = np.float32(ba2 / t) + np.float32(0.5 / t) * (
        (cba + ccj) * np.repeat(maskA, 2) + (cba + cci) * np.repeat(maskB, 2))

    sel8 = np.zeros((128, 32), np.float32)
    for m in range(8):
        sel8[32 * (m // 2) + (m % 2), m] = 1.0
    sel2 = np.zeros((128, 32), np.float32)
    for c in range(4):
        for mp in range(8):
            sel2[32 * c + mp, 8 * c + mp] = 1.0

    common = {
        "sel8": sel8.astype(ml_dtypes.bfloat16),
        "sel2": sel2.astype(ml_dtypes.bfloat16),
        "xT": xT.astype(ml_dtypes.bfloat16),
        "w1": W1.astype(ml_dtypes.bfloat16),
        "w2": W2.astype(ml_dtypes.bfloat16),
        "wa1lo": wa1lo.astype(ml_dtypes.bfloat16),
        "wa1hi": wa1hi.astype(ml_dtypes.bfloat16),
        "wa2c": wa2c.astype(ml_dtypes.bfloat16),
        "b1c": b1.reshape(H, 1),
        "b2c": b2.reshape(H, 1),
        "ba12": ba12.astype(np.float32),
        "scol": scol,
        "sbcol": sbcol2.reshape(128, 1).astype(np.float32),
        "wc2": wcomb2.astype(ml_dtypes.bfloat16),
        "wcej": wcej.astype(ml_dtypes.bfloat16),
        "wcei": wcei.astype(ml_dtypes.bfloat16),
        "bacj": bacj,
        "baci": baci,
        "mc": mcol,
    }
    in_maps = []
    for c in range(NCORES):
        m = dict(common)
        m["xbT"] = np.ascontiguousarray(
            xT[:, c * BLK:(c + 1) * BLK]).astype(ml_dtypes.bfloat16)
        in_maps.append(m)
    return in_maps


def kernel(node_features, W1, b1, W2, b2, Wa1, ba1, wa2, ba2, temperature):
    from concourse.bass_utils import run_bass_kernel_spmd

    if "nc" not in _cache:
        _cache["nc"] = _build_program()
    nc = _cache["nc"]

    in_maps = _host_prep(node_features, W1, b1, W2, b2, Wa1, ba1, wa2, ba2,
                         temperature)
    res = run_bass_kernel_spmd(nc, in_maps, list(range(NCORES)))
    adj = np.concatenate([res.results[c]["adj_blk"] for c in range(NCORES)],
                         axis=0)
    loss = np.float32(SPARSITY_WEIGHT) * np.mean(np.abs(adj), dtype=np.float32)
    return adj, np.float32(loss)


# revision 35
# speedup vs baseline: 1.0764x; 1.0486x over previous
"""Trainium2 Bass kernel for EnhancedGraphGenerator (GAT-style pairwise scorer).

Math (reference):
    h   = relu(x @ W1 + b1) @ W2 + b2                       # [N, H]
    e_i = h @ Wa1[:H]; e_j = h @ Wa1[H:]                    # [N, H]
    z   = relu(e_i[:,None,:] + e_j[None,:,:] + ba1)         # [N, N, H]
    s   = z . wa2 + ba2; s /= clip(t, .1, 5); s = (s+s.T)/2
    adj = sigmoid(s);  loss = 0.01 * mean(|adj|)

Distribution: row-sharded across 8 cores; h/e replicated (cheap), each core
computes both (i,j) and (j,i) raw scores for its 128 rows locally.

Device layout: h on partitions (64, duplicated to 128 for an i-pair), node
index on free dim. Pre-activation relu(e_jT + bias_col) is one fused
tensor_scalar/activation op over [128, 1024] in bf16 (DVE 4x mode); spread
over DVE/GPSIMD/ACT. The wa2 dot is a bf16 PE matmul with stationary
[128, 32] weights (wa2 in cols 0/1, zeros elsewhere -> psum rows 2..31
zeroed for free). tile_position col-tiling packs 4 i-pair blocks per PSUM
bank pair; pass A starts, pass B accumulates (same col group -> same bank
partitions, no cross-group has_written hazard). One ACT sigmoid per quad
evacuates into a persistent sig_all staging tile; 8 big strided DMAs write
the final adjacency rows.
"""

import sys

sys.path.insert(0, "/opt/trn_rl_repo")

import numpy as np

N, F, H = 1024, 512, 64
NCORES = 8
BLK = N // NCORES  # 128 rows per core
SPARSITY_WEIGHT = 0.01

_cache = {}

# producer engine schedule (weighted round robin): v=DVE, g=GPSIMD, a=ACT
N_PROD, N_ACT = 128, 20
PROD_PATTERN = "".join(
    "a" if (i * N_ACT) // N_PROD != ((i + 1) * N_ACT) // N_PROD else "v"
    for i in range(N_PROD))


def _build_program():
    import concourse.bacc as bacc
    import concourse.tile as tile
    from concourse import mybir

    f32 = mybir.dt.float32
    bf16 = mybir.dt.bfloat16
    AF = mybir.ActivationFunctionType
    OP = mybir.AluOpType

    nc = bacc.Bacc("TRN2", target_bir_lowering=False, debug=False)

    # ---- DRAM I/O ----
    xT_d = nc.dram_tensor("xT", [F, N], bf16, kind="ExternalInput")
    xbT_d = nc.dram_tensor("xbT", [F, BLK], bf16, kind="ExternalInput")
    w1_d = nc.dram_tensor("w1", [F, H], bf16, kind="ExternalInput")
    w2_d = nc.dram_tensor("w2", [H, H], bf16, kind="ExternalInput")
    wa1lo_d = nc.dram_tensor("wa1lo", [H, 128], bf16, kind="ExternalInput")
    wa1hi_d = nc.dram_tensor("wa1hi", [H, 128], bf16, kind="ExternalInput")
    wa2c_d = nc.dram_tensor("wa2c", [128, 32], bf16, kind="ExternalInput")
    b1_d = nc.dram_tensor("b1c", [H, 1], f32, kind="ExternalInput")
    b2_d = nc.dram_tensor("b2c", [H, 1], f32, kind="ExternalInput")
    ba12_d = nc.dram_tensor("ba12", [128, 1], f32, kind="ExternalInput")
    bacj_d = nc.dram_tensor("bacj", [128, 1], f32, kind="ExternalInput")
    baci_d = nc.dram_tensor("baci", [128, 1], f32, kind="ExternalInput")
    scol_d = nc.dram_tensor("scol", [128, 1], f32, kind="ExternalInput")
    sbcol_d = nc.dram_tensor("sbcol", [128, 1], f32, kind="ExternalInput")
    wcej_d = nc.dram_tensor("wcej", [H, 128], bf16, kind="ExternalInput")
    wcei_d = nc.dram_tensor("wcei", [H, 128], bf16, kind="ExternalInput")
    wc2_d = nc.dram_tensor("wc2", [H, 2], bf16, kind="ExternalInput")
    mc_d = nc.dram_tensor("mc", [128, 2], f32, kind="ExternalInput")
    sel8_d = nc.dram_tensor("sel8", [128, 32], bf16, kind="ExternalInput")
    sel2_d = nc.dram_tensor("sel2", [128, 32], bf16, kind="ExternalInput")
    adj_d = nc.dram_tensor("adj_blk", [BLK, N], f32, kind="ExternalOutput")

    with tile.TileContext(nc) as tc:
        with (
            tc.tile_pool(name="consts", bufs=1) as cp,
            tc.tile_pool(name="pre", bufs=12) as pp,
        ):
            ps0_cm = tc.tile_pool(name="psum0", bufs=2, space="PSUM")
            ps0 = ps0_cm.__enter__()
            # ---- const / input loads (ordered by first use; the two
            # DMA queues serialize, so gating transfers go first) ----
            w1t = cp.tile([128, 4 * H], bf16)
            for c in range(4):
                nc.sync.dma_start(w1t[:, c * H:(c + 1) * H],
                                  w1_d[c * 128:(c + 1) * 128, :])
            b1c = cp.tile([H, 1], f32)
            nc.gpsimd.dma_start(b1c[:], b1_d[:])
            wcej = cp.tile([H, 128], bf16)
            nc.gpsimd.dma_start(wcej[:], wcej_d[:])
            wcei = cp.tile([H, 128], bf16)
            nc.gpsimd.dma_start(wcei[:], wcei_d[:])
            xt = cp.tile([128, 4 * N], bf16)
            for c in range(4):
                for hh in range(2):
                    eng = nc.sync if (2 * c + hh) % 2 == 0 else nc.gpsimd
                    eng.dma_start(
                        xt[:, c * N + hh * 512:c * N + (hh + 1) * 512],
                        xT_d[c * 128:(c + 1) * 128, hh * 512:(hh + 1) * 512])
            xbt = cp.tile([128, 4 * BLK], bf16)
            for c in range(4):
                eng = nc.sync if c % 2 == 0 else nc.gpsimd
                eng.dma_start(xbt[:, c * BLK:(c + 1) * BLK],
                              xbT_d[c * 128:(c + 1) * 128, :])
            w2t = cp.tile([H, H], bf16)
            nc.sync.dma_start(w2t[:], w2_d[:])
            b2c = cp.tile([H, 1], f32)
            nc.gpsimd.dma_start(b2c[:], b2_d[:])
            wa1lo = cp.tile([H, 128], bf16)
            nc.sync.dma_start(wa1lo[:], wa1lo_d[:])
            wa1hi = cp.tile([H, 128], bf16)
            nc.gpsimd.dma_start(wa1hi[:], wa1hi_d[:])
            bacj = cp.tile([128, 1], f32)
            nc.sync.dma_start(bacj[:], bacj_d[:])
            baci = cp.tile([128, 1], f32)
            nc.gpsimd.dma_start(baci[:], baci_d[:])
            wa2c = cp.tile([128, 32], bf16)
            nc.sync.dma_start(wa2c[:], wa2c_d[:])
            ba12 = cp.tile([128, 1], f32)
            nc.gpsimd.dma_start(ba12[:], ba12_d[:])
            scol = cp.tile([128, 1], f32)
            nc.sync.dma_start(scol[:], scol_d[:])
            sbcol = cp.tile([128, 1], f32)
            nc.gpsimd.dma_start(sbcol[:], sbcol_d[:])
            wc2 = cp.tile([H, 2], bf16)
            nc.sync.dma_start(wc2[:], wc2_d[:])
            mc = cp.tile([128, 2], f32)
            nc.gpsimd.dma_start(mc[:], mc_d[:])
            sel8 = cp.tile([128, 32], bf16)
            nc.sync.dma_start(sel8[:], sel8_d[:])
            sel2 = cp.tile([128, 32], bf16)
            nc.gpsimd.dma_start(sel2[:], sel2_d[:])

            # ---- stage 0, j-side: e_jT / e_iT over all N, duplicated 2x on
            # partitions (rows 0-63 and 64-127 both hold the 64 h-dims) ----
            h1T = cp.tile([H, N], bf16)
            ejT2 = cp.tile([128, N], bf16)
            eiT2 = cp.tile([128, N], bf16)
            for jh in range(2):
                s = slice(512 * jh, 512 * (jh + 1))
                ps1 = ps0.tile([H, 512], f32, tag="s0")
                for c in range(4):
                    nc.tensor.matmul(ps1[:], w1t[:, c * H:(c + 1) * H],
                                     xt[:, c * N + 512 * jh: c * N + 512 * (jh + 1)],
                                     start=(c == 0), stop=(c == 3))
                nc.scalar.activation(h1T[:, s], ps1[:], AF.Relu, bias=b1c[:])
                # e_jT = (W2 @ Wa1_hi_dup).T @ h1T  (+b2-term folded into the
                # bias columns on the other side of the pre-act sum)
                psj = ps0.tile([128, 512], f32, tag="s0")
                nc.tensor.matmul(psj[:], wcej[:], h1T[:, s], start=True, stop=True)
                nc.vector.tensor_copy(ejT2[:, s], psj[:])
                psi = ps0.tile([128, 512], f32, tag="s0")
                nc.tensor.matmul(psi[:], wcei[:], h1T[:, s], start=True, stop=True)
                nc.vector.tensor_copy(eiT2[:, s], psi[:])

            # ---- stage 0, i-side (this core's 128 rows, from xbT) ----
            h1Tb = cp.tile([H, BLK], bf16)
            hTb = cp.tile([H, BLK], bf16)
            eiTb2p = cp.tile([128, BLK], f32)
            ejTb2p = cp.tile([128, BLK], f32)
            psb1 = ps0.tile([H, BLK], f32, tag="s0")
            for c in range(4):
                nc.tensor.matmul(psb1[:], w1t[:, c * H:(c + 1) * H],
                                 xbt[:, c * BLK:(c + 1) * BLK],
                                 start=(c == 0), stop=(c == 3))
            nc.scalar.activation(h1Tb[:], psb1[:], AF.Relu, bias=b1c[:])
            psb2 = ps0.tile([H, BLK], f32, tag="s0")
            nc.tensor.matmul(psb2[:], w2t[:], h1Tb[:], start=True, stop=True)
            nc.scalar.activation(hTb[:], psb2[:], AF.Identity, bias=b2c[:])
            psbe = ps0.tile([128, BLK], f32, tag="s0")
            nc.tensor.matmul(psbe[:], wa1lo[:], hTb[:], start=True, stop=True)
            nc.scalar.activation(eiTb2p[:], psbe[:], AF.Identity, bias=bacj[:])
            psbe2 = ps0.tile([128, BLK], f32, tag="s0")
            nc.tensor.matmul(psbe2[:], wa1hi[:], hTb[:], start=True, stop=True)
            nc.scalar.activation(ejTb2p[:], psbe2[:], AF.Identity, bias=baci[:])

            # bias columns: biasA[:, b] = [e_i(2b)+ba1 ; e_i(2b+1)+ba1]
            # (positive for ACT relu-form, negated for DVE max-form:
            #  relu(e+b) = max(e,-b) + b, the +b folds into a per-row
            #  constant applied at the final sigmoid)
            biasA = cp.tile([128, 64], f32)
            biasB = cp.tile([128, 64], f32)
            nc.vector.tensor_copy(biasA[0:64, :], eiTb2p[0:64, 0:128:2])
            nc.vector.tensor_copy(biasA[64:128, :], eiTb2p[64:128, 1:128:2])
            nc.vector.tensor_copy(biasB[0:64, :], ejTb2p[0:64, 0:128:2])
            nc.vector.tensor_copy(biasB[64:128, :], ejTb2p[64:128, 1:128:2])
            nbiasA = cp.tile([128, 64], f32)
            nbiasB = cp.tile([128, 64], f32)
            nc.vector.tensor_scalar(nbiasA[:], biasA[:], -1.0, None, OP.mult)
            nc.vector.tensor_scalar(nbiasB[:], biasB[:], -1.0, None, OP.mult)

            # per-row correction C_i = sum_h wa2[h]*(e_i+ba1) / (e_j+ba1),
            # masked by which pass used the max-form, times scale, plus
            # sigmoid bias -> final per-partition bias column
            psc2 = ps0.tile([128, 2], f32, tag="s0")
            nc.tensor.matmul(psc2[:], hTb[:], wc2[:], start=True, stop=True)
            cc2 = cp.tile([128, 2], f32)
            nc.vector.tensor_mul(cc2[:], psc2[:], mc[:])
            fb1 = cp.tile([128, 1], f32)
            nc.vector.tensor_add(fb1[:], cc2[:, 0:1], cc2[:, 1:2])
            fb2 = cp.tile([128, 1], f32)
            nc.vector.tensor_scalar(fb2[:], fb1[:], scol[:], sbcol[:],
                                    OP.mult, OP.add)

            ps0_cm.__exit__(None, None, None)
            psM_cm = tc.tile_pool(name="psumM", bufs=2, space="PSUM")
            psM = psM_cm.__enter__()
            psC_cm = tc.tile_pool(name="psumC", bufs=2, space="PSUM")
            psC = psC_cm.__enter__()

            # ---- main loop: 16 quads x 4 col-groups x (A,B) passes ----
            prod_idx = 0

            def producer(out_t, in_t, pos_col, neg_col):
                nonlocal prod_idx
                kind = PROD_PATTERN[prod_idx % len(PROD_PATTERN)]
                if kind == "a":
                    nc.scalar.activation(out_t[:], in_t[:], AF.Relu,
                                         bias=pos_col)
                else:
                    nc.vector.tensor_scalar(out_t[:], in_t[:], neg_col, None,
                                            OP.max)
                prod_idx += 1

            # compaction: stage-1 gathers each quad's 8 scattered psum rows
            # (partitions {32g, 32g+1}) to rows 32c..32c+8 of CP[a] where
            # q = 4a + c; stage-2 gathers CP's 4x8 rows into 32 contiguous
            # rows per col group -> fully i-ordered CP2 [128, 1024]
            CS = cp.tile([128, 4 * N], bf16)
            adj_s = cp.tile([128, N], f32)
            cp2 = psC.tile([128, N], f32, tag="cp", name="cp2")
            state = {"cpa": None, "ready_a": None}

            def finalize_a(a):
                # stage-2 compaction + sigmoid + output DMA for one group of
                # 32 adjacency rows (fully i-ordered)
                for jh in range(2):
                    s = slice(512 * jh, 512 * (jh + 1))
                    nc.tensor.matmul(cp2[32 * a:32 * a + 32, s], sel2[:],
                                     CS[:, a * N + 512 * jh:
                                         a * N + 512 * (jh + 1)],
                                     start=True, stop=True,
                                     tile_position=(0, 32 * a),
                                     skip_group_check=True)
                r = slice(32 * a, 32 * a + 32)
                nc.scalar.activation(adj_s[r, :], cp2[r, :], AF.Sigmoid,
                                     bias=fb2[r, :], scale=scol[r, :])
                eng = nc.sync if a % 2 == 0 else nc.gpsimd
                eng.dma_start(adj_d[r, :], adj_s[r, :])

            def epilogue(q, pq):
                # runs one quad behind the producer/MM front so the FIFO
                # engine queues never stall on cross-engine dependencies
                a, c = q // 4, q % 4
                sig = pp.tile([128, N], bf16, tag="sig")
                nc.scalar.copy(sig[:], pq[:])
                if c == 0:
                    state["cpa"] = psC.tile([128, N], f32, tag="cp", name="cpa")
                cpa = state["cpa"]
                for jh in range(2):
                    s = slice(512 * jh, 512 * (jh + 1))
                    nc.tensor.matmul(cpa[32 * c:32 * c + 32, s], sel8[:],
                                     sig[:, s], start=True, stop=True,
                                     tile_position=(0, 32 * c),
                                     skip_group_check=True)
                if state["ready_a"] is not None:
                    finalize_a(state["ready_a"])
                    state["ready_a"] = None
                if c == 3:
                    if a % 2 == 0:
                        nc.vector.tensor_copy(CS[:, a * N:(a + 1) * N], cpa[:])
                    else:
                        nc.scalar.copy(CS[:, a * N:(a + 1) * N], cpa[:])
                    state["ready_a"] = a

            pending = None
            for q in range(16):
                pq = psM.tile([128, N], f32, tag="pq")
                pres = []
                for g in range(4):
                    b = 4 * q + g
                    preA = pp.tile([128, N], bf16, tag="pre")
                    producer(preA, ejT2, biasA[:, b:b + 1], nbiasA[:, b:b + 1])
                    preB = pp.tile([128, N], bf16, tag="pre")
                    producer(preB, eiT2, biasB[:, b:b + 1], nbiasB[:, b:b + 1])
                    pres.append((preA, preB))
                if pending is not None:
                    epilogue(*pending)
                # sim's group tracker is partition-base blind -> false
                # collisions across col groups; HW has_written is
                # per-partition and each MM consumes its whole zero
                # region, so skipping the check is sound here
                for jh in range(2):
                    s = slice(512 * jh, 512 * (jh + 1))
                    for g in range(4):
                        nc.tensor.matmul(pq[32 * g:32 * g + 32, s], wa2c[:],
                                         pres[g][0][:, s], start=True,
                                         stop=False, tile_position=(0, 32 * g),
                                         skip_group_check=True)
                for jh in range(2):
                    s = slice(512 * jh, 512 * (jh + 1))
                    for g in range(4):
                        nc.tensor.matmul(pq[32 * g:32 * g + 32, s], wa2c[:],
                                         pres[g][1][:, s], start=False,
                                         stop=True, tile_position=(0, 32 * g),
                                         skip_group_check=True)
                pending = (q, pq)
            epilogue(*pending)

            if state["ready_a"] is not None:
                finalize_a(state["ready_a"])
            psC_cm.__exit__(None, None, None)
            psM_cm.__exit__(None, None, None)

    nc.compile()
    return nc


def _host_prep(node_features, W1, b1, W2, b2, Wa1, ba1, wa2, ba2, temperature):
    """Host-side input layout prep (transposes / tiling / scalar folding)."""
    import ml_dtypes

    x = np.asarray(node_features, np.float32)
    W1 = np.asarray(W1, np.float32)
    W2 = np.asarray(W2, np.float32)
    Wa1 = np.asarray(Wa1, np.float32)
    b1 = np.asarray(b1, np.float32)
    b2 = np.asarray(b2, np.float32)
    ba1 = np.asarray(ba1, np.float32)
    wa2 = np.asarray(wa2, np.float32)
    ba2 = np.float32(ba2)
    t = float(np.clip(np.float32(temperature), 0.1, 5.0))

    xT = np.ascontiguousarray(x.T)                           # [F, N]
    wa1lo = np.ascontiguousarray(np.tile(Wa1[:H], (1, 2)))   # [64, 128]
    wa1hi = np.ascontiguousarray(np.tile(Wa1[H:], (1, 2)))   # [64, 128]
    wa2c = np.zeros((128, 32), np.float32)
    wa2c[0:64, 0] = wa2
    wa2c[64:128, 1] = wa2
    ba12 = np.tile(ba1, 2).reshape(128, 1)
    scol = np.full((128, 1), 0.5 / t, np.float32)
    sbcol = np.full((128, 1), ba2 / t, np.float32)

    wcej = np.ascontiguousarray(W2 @ np.tile(Wa1[H:], (1, 2)))  # [64, 128]
    wcei = np.ascontiguousarray(W2 @ np.tile(Wa1[:H], (1, 2)))
    cj = np.tile(Wa1[H:].T @ b2, 2)    # b2-term of e_j, dup layout
    ci = np.tile(Wa1[:H].T @ b2, 2)
    bacj = (np.tile(ba1, 2) + cj).reshape(128, 1).astype(np.float32)
    baci = (np.tile(ba1, 2) + ci).reshape(128, 1).astype(np.float32)
    wcomb2 = np.stack([Wa1[:H] @ wa2, Wa1[H:] @ wa2], axis=1)  # [64, 2]
    cba = float(wa2 @ ba1)
    maskA = np.array([1.0 if PROD_PATTERN[(2 * b) % len(PROD_PATTERN)] == "v"
                      else 0.0 for b in range(64)], np.float32)
    maskB = np.array([1.0 if PROD_PATTERN[(2 * b + 1) % len(PROD_PATTERN)] == "v"
                      else 0.0 for b in range(64)], np.float32)
    mcol = np.zeros((128, 2), np.float32)
    mcol[:, 0] = np.repeat(maskA, 2)
    mcol[:, 1] = np.repeat(maskB, 2)
    ccj = float(wa2 @ (Wa1[H:].T @ b2))   # b2-term of pass-A bias fold
    cci = float(wa2 @ (Wa1[:H].T @ b2))   # b2-term of pass-B bias fold
    sbcol2 = np.float32(ba2 / t) + np.float32(0.5 / t) * (
        (cba + ccj) * np.repeat(maskA, 2) + (cba + cci) * np.repeat(maskB, 2))

    sel8 = np.zeros((128, 32), np.float32)
    for m in range(8):
        sel8[32 * (m // 2) + (m % 2), m] = 1.0
    sel2 = np.zeros((128, 32), np.float32)
    for c in range(4):
        for mp in range(8):
            sel2[32 * c + mp, 8 * c + mp] = 1.0

    common = {
        "sel8": sel8.astype(ml_dtypes.bfloat16),
        "sel2": sel2.astype(ml_dtypes.bfloat16),
        "xT": xT.astype(ml_dtypes.bfloat16),
        "w1": W1.astype(ml_dtypes.bfloat16),
        "w2": W2.astype(ml_dtypes.bfloat16),
        "wa1lo": wa1lo.astype(ml_dtypes.bfloat16),
        "wa1hi": wa1hi.astype(ml_dtypes.bfloat16),
        "wa2c": wa2c.astype(ml_dtypes.bfloat16),
        "b1c": b1.reshape(H, 1),
        "b2c": b2.reshape(H, 1),
        "ba12": ba12.astype(np.float32),
        "scol": scol,
        "sbcol": sbcol2.reshape(128, 1).astype(np.float32),
        "wc2": wcomb2.astype(ml_dtypes.bfloat16),
        "wcej": wcej.astype(ml_dtypes.bfloat16),
        "wcei": wcei.astype(ml_dtypes.bfloat16),
        "bacj": bacj,
        "baci": baci,
        "mc": mcol,
    }
    in_maps = []
    for c in range(NCORES):
        m = dict(common)
        m["xbT"] = np.ascontiguousarray(
            xT[:, c * BLK:(c + 1) * BLK]).astype(ml_dtypes.bfloat16)
        in_maps.append(m)
    return in_maps


def kernel(node_features, W1, b1, W2, b2, Wa1, ba1, wa2, ba2, temperature):
    from concourse.bass_utils import run_bass_kernel_spmd

    if "nc" not in _cache:
        _cache["nc"] = _build_program()
    nc = _cache["nc"]

    in_maps = _host_prep(node_features, W1, b1, W2, b2, Wa1, ba1, wa2, ba2,
                         temperature)
    res = run_bass_kernel_spmd(nc, in_maps, list(range(NCORES)))
    adj = np.concatenate([res.results[c]["adj_blk"] for c in range(NCORES)],
                         axis=0)
    loss = np.float32(SPARSITY_WEIGHT) * np.mean(np.abs(adj), dtype=np.float32)
    return adj, np.float32(loss)


# revision 36
# speedup vs baseline: 1.0877x; 1.0105x over previous
"""Trainium2 Bass kernel for EnhancedGraphGenerator (GAT-style pairwise scorer).

Math (reference):
    h   = relu(x @ W1 + b1) @ W2 + b2                       # [N, H]
    e_i = h @ Wa1[:H]; e_j = h @ Wa1[H:]                    # [N, H]
    z   = relu(e_i[:,None,:] + e_j[None,:,:] + ba1)         # [N, N, H]
    s   = z . wa2 + ba2; s /= clip(t, .1, 5); s = (s+s.T)/2
    adj = sigmoid(s);  loss = 0.01 * mean(|adj|)

Distribution: row-sharded across 8 cores; h/e replicated (cheap), each core
computes both (i,j) and (j,i) raw scores for its 128 rows locally.

Device layout: h on partitions (64, duplicated to 128 for an i-pair), node
index on free dim. Pre-activation relu(e_jT + bias_col) is one fused
tensor_scalar/activation op over [128, 1024] in bf16 (DVE 4x mode); spread
over DVE/GPSIMD/ACT. The wa2 dot is a bf16 PE matmul with stationary
[128, 32] weights (wa2 in cols 0/1, zeros elsewhere -> psum rows 2..31
zeroed for free). tile_position col-tiling packs 4 i-pair blocks per PSUM
bank pair; pass A starts, pass B accumulates (same col group -> same bank
partitions, no cross-group has_written hazard). One ACT sigmoid per quad
evacuates into a persistent sig_all staging tile; 8 big strided DMAs write
the final adjacency rows.
"""

import sys

sys.path.insert(0, "/opt/trn_rl_repo")

import numpy as np

N, F, H = 1024, 512, 64
NCORES = 8
BLK = N // NCORES  # 128 rows per core
SPARSITY_WEIGHT = 0.01

_cache = {}

# producer engine schedule (weighted round robin): v=DVE, g=GPSIMD, a=ACT
N_PROD, N_ACT = 128, 16
PROD_PATTERN = "".join(
    "a" if (i * N_ACT) // N_PROD != ((i + 1) * N_ACT) // N_PROD else "v"
    for i in range(N_PROD))


def _build_program():
    import concourse.bacc as bacc
    import concourse.tile as tile
    from concourse import mybir

    f32 = mybir.dt.float32
    bf16 = mybir.dt.bfloat16
    AF = mybir.ActivationFunctionType
    OP = mybir.AluOpType

    nc = bacc.Bacc("TRN2", target_bir_lowering=False, debug=False)

    # ---- DRAM I/O ----
    xT_d = nc.dram_tensor("xT", [F, N], bf16, kind="ExternalInput")
    xbT_d = nc.dram_tensor("xbT", [F, BLK], bf16, kind="ExternalInput")
    w1_d = nc.dram_tensor("w1", [F, H], bf16, kind="ExternalInput")
    w2_d = nc.dram_tensor("w2", [H, H], bf16, kind="ExternalInput")
    wa1lo_d = nc.dram_tensor("wa1lo", [H, 128], bf16, kind="ExternalInput")
    wa1hi_d = nc.dram_tensor("wa1hi", [H, 128], bf16, kind="ExternalInput")
    wa2c_d = nc.dram_tensor("wa2c", [128, 32], bf16, kind="ExternalInput")
    b1_d = nc.dram_tensor("b1c", [H, 1], f32, kind="ExternalInput")
    b2_d = nc.dram_tensor("b2c", [H, 1], f32, kind="ExternalInput")
    ba12_d = nc.dram_tensor("ba12", [128, 1], f32, kind="ExternalInput")
    bacj_d = nc.dram_tensor("bacj", [128, 1], f32, kind="ExternalInput")
    baci_d = nc.dram_tensor("baci", [128, 1], f32, kind="ExternalInput")
    scol_d = nc.dram_tensor("scol", [128, 1], f32, kind="ExternalInput")
    sbcol_d = nc.dram_tensor("sbcol", [128, 1], f32, kind="ExternalInput")
    wcej_d = nc.dram_tensor("wcej", [H, 128], bf16, kind="ExternalInput")
    wcei_d = nc.dram_tensor("wcei", [H, 128], bf16, kind="ExternalInput")
    wc2_d = nc.dram_tensor("wc2", [H, 2], bf16, kind="ExternalInput")
    mc_d = nc.dram_tensor("mc", [128, 2], f32, kind="ExternalInput")
    sel8_d = nc.dram_tensor("sel8", [128, 32], bf16, kind="ExternalInput")
    sel2_d = nc.dram_tensor("sel2", [128, 32], bf16, kind="ExternalInput")
    adj_d = nc.dram_tensor("adj_blk", [BLK, N], f32, kind="ExternalOutput")

    with tile.TileContext(nc) as tc:
        with (
            tc.tile_pool(name="consts", bufs=1) as cp,
            tc.tile_pool(name="pre", bufs=12) as pp,
        ):
            ps0_cm = tc.tile_pool(name="psum0", bufs=2, space="PSUM")
            ps0 = ps0_cm.__enter__()
            # ---- const / input loads (ordered by first use; the two
            # DMA queues serialize, so gating transfers go first) ----
            w1t = cp.tile([128, 4 * H], bf16)
            for c in range(4):
                nc.sync.dma_start(w1t[:, c * H:(c + 1) * H],
                                  w1_d[c * 128:(c + 1) * 128, :])
            b1c = cp.tile([H, 1], f32)
            nc.gpsimd.dma_start(b1c[:], b1_d[:])
            wcej = cp.tile([H, 128], bf16)
            nc.gpsimd.dma_start(wcej[:], wcej_d[:])
            wcei = cp.tile([H, 128], bf16)
            nc.gpsimd.dma_start(wcei[:], wcei_d[:])
            xt = cp.tile([128, 4 * N], bf16)
            for c in range(4):
                for hh in range(2):
                    eng = nc.sync if (2 * c + hh) % 2 == 0 else nc.gpsimd
                    eng.dma_start(
                        xt[:, c * N + hh * 512:c * N + (hh + 1) * 512],
                        xT_d[c * 128:(c + 1) * 128, hh * 512:(hh + 1) * 512])
            xbt = cp.tile([128, 4 * BLK], bf16)
            for c in range(4):
                eng = nc.sync if c % 2 == 0 else nc.gpsimd
                eng.dma_start(xbt[:, c * BLK:(c + 1) * BLK],
                              xbT_d[c * 128:(c + 1) * 128, :])
            w2t = cp.tile([H, H], bf16)
            nc.sync.dma_start(w2t[:], w2_d[:])
            b2c = cp.tile([H, 1], f32)
            nc.gpsimd.dma_start(b2c[:], b2_d[:])
            wa1lo = cp.tile([H, 128], bf16)
            nc.sync.dma_start(wa1lo[:], wa1lo_d[:])
            wa1hi = cp.tile([H, 128], bf16)
            nc.gpsimd.dma_start(wa1hi[:], wa1hi_d[:])
            bacj = cp.tile([128, 1], f32)
            nc.sync.dma_start(bacj[:], bacj_d[:])
            baci = cp.tile([128, 1], f32)
            nc.gpsimd.dma_start(baci[:], baci_d[:])
            wa2c = cp.tile([128, 32], bf16)
            nc.sync.dma_start(wa2c[:], wa2c_d[:])
            ba12 = cp.tile([128, 1], f32)
            nc.gpsimd.dma_start(ba12[:], ba12_d[:])
            scol = cp.tile([128, 1], f32)
            nc.sync.dma_start(scol[:], scol_d[:])
            sbcol = cp.tile([128, 1], f32)
            nc.gpsimd.dma_start(sbcol[:], sbcol_d[:])
            wc2 = cp.tile([H, 2], bf16)
            nc.sync.dma_start(wc2[:], wc2_d[:])
            mc = cp.tile([128, 2], f32)
            nc.gpsimd.dma_start(mc[:], mc_d[:])
            sel8 = cp.tile([128, 32], bf16)
            nc.sync.dma_start(sel8[:], sel8_d[:])
            sel2 = cp.tile([128, 32], bf16)
            nc.gpsimd.dma_start(sel2[:], sel2_d[:])

            # ---- stage 0, j-side: e_jT / e_iT over all N, duplicated 2x on
            # partitions (rows 0-63 and 64-127 both hold the 64 h-dims) ----
            h1T = cp.tile([H, N], bf16)
            ejT2 = cp.tile([128, N], bf16)
            eiT2 = cp.tile([128, N], bf16)
            for jh in range(2):
                s = slice(512 * jh, 512 * (jh + 1))
                ps1 = ps0.tile([H, 512], f32, tag="s0")
                for c in range(4):
                    nc.tensor.matmul(ps1[:], w1t[:, c * H:(c + 1) * H],
                                     xt[:, c * N + 512 * jh: c * N + 512 * (jh + 1)],
                                     start=(c == 0), stop=(c == 3))
                nc.scalar.activation(h1T[:, s], ps1[:], AF.Relu, bias=b1c[:])
                # e_jT = (W2 @ Wa1_hi_dup).T @ h1T  (+b2-term folded into the
                # bias columns on the other side of the pre-act sum)
                psj = ps0.tile([128, 512], f32, tag="s0")
                nc.tensor.matmul(psj[:], wcej[:], h1T[:, s], start=True, stop=True)
                nc.vector.tensor_copy(ejT2[:, s], psj[:])
                psi = ps0.tile([128, 512], f32, tag="s0")
                nc.tensor.matmul(psi[:], wcei[:], h1T[:, s], start=True, stop=True)
                nc.vector.tensor_copy(eiT2[:, s], psi[:])

            # ---- stage 0, i-side (this core's 128 rows, from xbT) ----
            h1Tb = cp.tile([H, BLK], bf16)
            hTb = cp.tile([H, BLK], bf16)
            eiTb2p = cp.tile([128, BLK], f32)
            ejTb2p = cp.tile([128, BLK], f32)
            psb1 = ps0.tile([H, BLK], f32, tag="s0")
            for c in range(4):
                nc.tensor.matmul(psb1[:], w1t[:, c * H:(c + 1) * H],
                                 xbt[:, c * BLK:(c + 1) * BLK],
                                 start=(c == 0), stop=(c == 3))
            nc.scalar.activation(h1Tb[:], psb1[:], AF.Relu, bias=b1c[:])
            psb2 = ps0.tile([H, BLK], f32, tag="s0")
            nc.tensor.matmul(psb2[:], w2t[:], h1Tb[:], start=True, stop=True)
            nc.scalar.activation(hTb[:], psb2[:], AF.Identity, bias=b2c[:])
            psbe = ps0.tile([128, BLK], f32, tag="s0")
            nc.tensor.matmul(psbe[:], wa1lo[:], hTb[:], start=True, stop=True)
            nc.scalar.activation(eiTb2p[:], psbe[:], AF.Identity, bias=bacj[:])
            psbe2 = ps0.tile([128, BLK], f32, tag="s0")
            nc.tensor.matmul(psbe2[:], wa1hi[:], hTb[:], start=True, stop=True)
            nc.scalar.activation(ejTb2p[:], psbe2[:], AF.Identity, bias=baci[:])

            # bias columns: biasA[:, b] = [e_i(2b)+ba1 ; e_i(2b+1)+ba1]
            # (positive for ACT relu-form, negated for DVE max-form:
            #  relu(e+b) = max(e,-b) + b, the +b folds into a per-row
            #  constant applied at the final sigmoid)
            biasA = cp.tile([128, 64], f32)
            biasB = cp.tile([128, 64], f32)
            nc.vector.tensor_copy(biasA[0:64, :], eiTb2p[0:64, 0:128:2])
            nc.vector.tensor_copy(biasA[64:128, :], eiTb2p[64:128, 1:128:2])
            nc.vector.tensor_copy(biasB[0:64, :], ejTb2p[0:64, 0:128:2])
            nc.vector.tensor_copy(biasB[64:128, :], ejTb2p[64:128, 1:128:2])
            nbiasA = cp.tile([128, 64], f32)
            nbiasB = cp.tile([128, 64], f32)
            nc.vector.tensor_scalar(nbiasA[:], biasA[:], -1.0, None, OP.mult)
            nc.vector.tensor_scalar(nbiasB[:], biasB[:], -1.0, None, OP.mult)

            # per-row correction C_i = sum_h wa2[h]*(e_i+ba1) / (e_j+ba1),
            # masked by which pass used the max-form, times scale, plus
            # sigmoid bias -> final per-partition bias column
            psc2 = ps0.tile([128, 2], f32, tag="s0")
            nc.tensor.matmul(psc2[:], hTb[:], wc2[:], start=True, stop=True)
            cc2 = cp.tile([128, 2], f32)
            nc.vector.tensor_mul(cc2[:], psc2[:], mc[:])
            fb1 = cp.tile([128, 1], f32)
            nc.vector.tensor_add(fb1[:], cc2[:, 0:1], cc2[:, 1:2])
            fb2 = cp.tile([128, 1], f32)
            nc.vector.tensor_scalar(fb2[:], fb1[:], scol[:], sbcol[:],
                                    OP.mult, OP.add)

            ps0_cm.__exit__(None, None, None)
            psM_cm = tc.tile_pool(name="psumM", bufs=2, space="PSUM")
            psM = psM_cm.__enter__()
            psC_cm = tc.tile_pool(name="psumC", bufs=2, space="PSUM")
            psC = psC_cm.__enter__()

            # ---- main loop: 16 quads x 4 col-groups x (A,B) passes ----
            prod_idx = 0

            def producer(out_t, in_t, pos_col, neg_col):
                nonlocal prod_idx
                kind = PROD_PATTERN[prod_idx % len(PROD_PATTERN)]
                if kind == "a":
                    nc.scalar.activation(out_t[:], in_t[:], AF.Relu,
                                         bias=pos_col)
                else:
                    nc.vector.tensor_scalar(out_t[:], in_t[:], neg_col, None,
                                            OP.max)
                prod_idx += 1

            # compaction: stage-1 gathers each quad's 8 scattered psum rows
            # (partitions {32g, 32g+1}) to rows 32c..32c+8 of CP[a] where
            # q = 4a + c; stage-2 gathers CP's 4x8 rows into 32 contiguous
            # rows per col group -> fully i-ordered CP2 [128, 1024]
            CS = cp.tile([128, 4 * N], bf16)
            adj_s = cp.tile([128, N], f32)
            cp2 = psC.tile([128, N], f32, tag="cp", name="cp2")
            state = {"cpa": None, "ready_a": None}

            def finalize_a(a):
                # stage-2 compaction + sigmoid + output DMA for one group of
                # 32 adjacency rows (fully i-ordered)
                for jh in range(2):
                    s = slice(512 * jh, 512 * (jh + 1))
                    nc.tensor.matmul(cp2[32 * a:32 * a + 32, s], sel2[:],
                                     CS[:, a * N + 512 * jh:
                                         a * N + 512 * (jh + 1)],
                                     start=True, stop=True,
                                     tile_position=(0, 32 * a),
                                     skip_group_check=True)
                r = slice(32 * a, 32 * a + 32)
                nc.scalar.activation(adj_s[r, :], cp2[r, :], AF.Sigmoid,
                                     bias=fb2[r, :], scale=scol[r, :])
                eng = nc.sync if a % 2 == 0 else nc.gpsimd
                eng.dma_start(adj_d[r, :], adj_s[r, :])

            def epilogue(q, pq):
                # runs one quad behind the producer/MM front so the FIFO
                # engine queues never stall on cross-engine dependencies
                a, c = q // 4, q % 4
                sig = pp.tile([128, N], bf16, tag="sig")
                nc.scalar.copy(sig[:], pq[:])
                if c == 0:
                    state["cpa"] = psC.tile([128, N], f32, tag="cp", name="cpa")
                cpa = state["cpa"]
                for jh in range(2):
                    s = slice(512 * jh, 512 * (jh + 1))
                    nc.tensor.matmul(cpa[32 * c:32 * c + 32, s], sel8[:],
                                     sig[:, s], start=True, stop=True,
                                     tile_position=(0, 32 * c),
                                     skip_group_check=True)
                if state["ready_a"] is not None:
                    finalize_a(state["ready_a"])
                    state["ready_a"] = None
                if c == 3:
                    if a % 2 == 0:
                        nc.vector.tensor_copy(CS[:, a * N:(a + 1) * N], cpa[:])
                    else:
                        nc.scalar.copy(CS[:, a * N:(a + 1) * N], cpa[:])
                    state["ready_a"] = a

            pending = None
            for q in range(16):
                pq = psM.tile([128, N], f32, tag="pq")
                pres = []
                for g in range(4):
                    b = 4 * q + g
                    preA = pp.tile([128, N], bf16, tag="pre")
                    producer(preA, ejT2, biasA[:, b:b + 1], nbiasA[:, b:b + 1])
                    preB = pp.tile([128, N], bf16, tag="pre")
                    producer(preB, eiT2, biasB[:, b:b + 1], nbiasB[:, b:b + 1])
                    pres.append((preA, preB))
                if pending is not None:
                    epilogue(*pending)
                # sim's group tracker is partition-base blind -> false
                # collisions across col groups; HW has_written is
                # per-partition and each MM consumes its whole zero
                # region, so skipping the check is sound here
                for jh in range(2):
                    s = slice(512 * jh, 512 * (jh + 1))
                    for g in range(4):
                        nc.tensor.matmul(pq[32 * g:32 * g + 32, s], wa2c[:],
                                         pres[g][0][:, s], start=True,
                                         stop=False, tile_position=(0, 32 * g),
                                         skip_group_check=True)
                for jh in range(2):
                    s = slice(512 * jh, 512 * (jh + 1))
                    for g in range(4):
                        nc.tensor.matmul(pq[32 * g:32 * g + 32, s], wa2c[:],
                                         pres[g][1][:, s], start=False,
                                         stop=True, tile_position=(0, 32 * g),
                                         skip_group_check=True)
                pending = (q, pq)
            epilogue(*pending)

            if state["ready_a"] is not None:
                finalize_a(state["ready_a"])
            psC_cm.__exit__(None, None, None)
            psM_cm.__exit__(None, None, None)

    nc.compile()
    return nc


def _host_prep(node_features, W1, b1, W2, b2, Wa1, ba1, wa2, ba2, temperature):
    """Host-side input layout prep (transposes / tiling / scalar folding)."""
    import ml_dtypes

    x = np.asarray(node_features, np.float32)
    W1 = np.asarray(W1, np.float32)
    W2 = np.asarray(W2, np.float32)
    Wa1 = np.asarray(Wa1, np.float32)
    b1 = np.asarray(b1, np.float32)
    b2 = np.asarray(b2, np.float32)
    ba1 = np.asarray(ba1, np.float32)
    wa2 = np.asarray(wa2, np.float32)
    ba2 = np.float32(ba2)
    t = float(np.clip(np.float32(temperature), 0.1, 5.0))

    xT = np.ascontiguousarray(x.T)                           # [F, N]
    wa1lo = np.ascontiguousarray(np.tile(Wa1[:H], (1, 2)))   # [64, 128]
    wa1hi = np.ascontiguousarray(np.tile(Wa1[H:], (1, 2)))   # [64, 128]
    wa2c = np.zeros((128, 32), np.float32)
    wa2c[0:64, 0] = wa2
    wa2c[64:128, 1] = wa2
    ba12 = np.tile(ba1, 2).reshape(128, 1)
    scol = np.full((128, 1), 0.5 / t, np.float32)
    sbcol = np.full((128, 1), ba2 / t, np.float32)

    wcej = np.ascontiguousarray(W2 @ np.tile(Wa1[H:], (1, 2)))  # [64, 128]
    wcei = np.ascontiguousarray(W2 @ np.tile(Wa1[:H], (1, 2)))
    cj = np.tile(Wa1[H:].T @ b2, 2)    # b2-term of e_j, dup layout
    ci = np.tile(Wa1[:H].T @ b2, 2)
    bacj = (np.tile(ba1, 2) + cj).reshape(128, 1).astype(np.float32)
    baci = (np.tile(ba1, 2) + ci).reshape(128, 1).astype(np.float32)
    wcomb2 = np.stack([Wa1[:H] @ wa2, Wa1[H:] @ wa2], axis=1)  # [64, 2]
    cba = float(wa2 @ ba1)
    maskA = np.array([1.0 if PROD_PATTERN[(2 * b) % len(PROD_PATTERN)] == "v"
                      else 0.0 for b in range(64)], np.float32)
    maskB = np.array([1.0 if PROD_PATTERN[(2 * b + 1) % len(PROD_PATTERN)] == "v"
                      else 0.0 for b in range(64)], np.float32)
    mcol = np.zeros((128, 2), np.float32)
    mcol[:, 0] = np.repeat(maskA, 2)
    mcol[:, 1] = np.repeat(maskB, 2)
    ccj = float(wa2 @ (Wa1[H:].T @ b2))   # b2-term of pass-A bias fold
    cci = float(wa2 @ (Wa1[:H].T @ b2))   # b2-term of pass-B bias fold
    sbcol2 = np.float32(ba2 / t) + np.float32(0.5 / t) * (
        (cba + ccj) * np.repeat(maskA, 2) + (cba + cci) * np.repeat(maskB, 2))

    sel8 = np.zeros((128, 32), np.float32)
    for m in range(8):
        sel8[32 * (m // 2) + (m % 2), m] = 1.0
    sel2 = np.zeros((128, 32), np.float32)
    for c in range(4):
        for mp in range(8):
            sel2[32 * c + mp, 8 * c + mp] = 1.0

    common = {
        "sel8": sel8.astype(ml_dtypes.bfloat16),
        "sel2": sel2.astype(ml_dtypes.bfloat16),
        "xT": xT.astype(ml_dtypes.bfloat16),
        "w1": W1.astype(ml_dtypes.bfloat16),
        "w2": W2.astype(ml_dtypes.bfloat16),
        "wa1lo": wa1lo.astype(ml_dtypes.bfloat16),
        "wa1hi": wa1hi.astype(ml_dtypes.bfloat16),
        "wa2c": wa2c.astype(ml_dtypes.bfloat16),
        "b1c": b1.reshape(H, 1),
        "b2c": b2.reshape(H, 1),
        "ba12": ba12.astype(np.float32),
        "scol": scol,
        "sbcol": sbcol2.reshape(128, 1).astype(np.float32),
        "wc2": wcomb2.astype(ml_dtypes.bfloat16),
        "wcej": wcej.astype(ml_dtypes.bfloat16),
        "wcei": wcei.astype(ml_dtypes.bfloat16),
        "bacj": bacj,
        "baci": baci,
        "mc": mcol,
    }
    in_maps = []
    for c in range(NCORES):
        m = dict(common)
        m["xbT"] = np.ascontiguousarray(
            xT[:, c * BLK:(c + 1) * BLK]).astype(ml_dtypes.bfloat16)
        in_maps.append(m)
    return in_maps


def kernel(node_features, W1, b1, W2, b2, Wa1, ba1, wa2, ba2, temperature):
    from concourse.bass_utils import run_bass_kernel_spmd

    if "nc" not in _cache:
        _cache["nc"] = _build_program()
    nc = _cache["nc"]

    in_maps = _host_prep(node_features, W1, b1, W2, b2, Wa1, ba1, wa2, ba2,
                         temperature)
    res = run_bass_kernel_spmd(nc, in_maps, list(range(NCORES)))
    adj = np.concatenate([res.results[c]["adj_blk"] for c in range(NCORES)],
                         axis=0)
    loss = np.float32(SPARSITY_WEIGHT) * np.mean(np.abs(adj), dtype=np.float32)
    return adj, np.float32(loss)
